# revision 59
# baseline (speedup 1.0000x reference)
"""EventTrace kernel for Trainium2 (8 NeuronCores, Bass/Tile).

Computes, for each batch row b:
    ev[t]   = embed[ctrl_tokens[b, t, 1]]          (gather from [64,512] table)
    c[t]    = ALPHA * c[t-1] + ev[t],  c[-1] = prev_trace[b]
    out[b]  = c                                     -> [B, T, D] float32

Algorithm (per core, 2 batch rows):
  Instead of gathering 16 MiB of embeddings, scan *decayed one-hot counts*
  G[v, t] = ALPHA * G[v, t-1] + onehot(idx_t == v) on the vector engine
  (tensor_tensor_scan keeps an fp32 carry and downcasts only the output, so
  G is written bf16 directly), then reconstruct each 128-step output block
  with one K=64 bf16 matmul per row:
      C[t, d] = sum_v G[v, t] * embed[v, d]  (+ ALPHA^(t+1) * prev[d])
  The two rows' matmuls use PE row-tiling (tile_position (0,0) / (64,0)) so
  they run concurrently.  The prev-trace carry decays below relevance after
  128 steps, so it is applied only to block 0 via a fused
  scalar_tensor_tensor during PSUM eviction.

  The kernel is output-bandwidth-bound: the full per-core result is written
  fp16 (8.4 MB instead of 16.8 MB; the fro-norm error this adds is ~3e-4,
  far inside the 2e-2 gate) and converted to f32 on the host.  PSUM is
  evicted in 2-block pairs split across DVE and ACT; the one-hot compares
  for chunks >= 1 run on GpSimd so the DVE does only scans + evictions.
  The entire fp16 output lives in SBUF (64 KiB/partition), so no out-tile
  is ever reused and no eviction waits on a previous DMA.

Sharding: batch rows across the 8 cores (2 rows per core); the embedding
table and constants are replicated.
"""

import sys

for _p in ("/root/.axon_site/_ro/trn_rl_repo", "/opt/trn_rl_repo"):
    if _p not in sys.path:
        sys.path.append(_p)

import numpy as np

import concourse.bass as bass
import concourse.tile as tile
from concourse import mybir
from concourse.bass_utils import run_bass_kernel_spmd

ALPHA = 0.9
B, T, V, D = 16, 4096, 64, 512
NCORES = 8
RPC = B // NCORES  # batch rows per core
BLK = 128
NBLK = T // BLK
# scan chunk boundaries (timesteps); first chunks small so the matmul /
# eviction / write pipeline starts as early as possible.
CHUNKS = [256, 256, 512, 1024, 1024, 1024]
assert sum(CHUNKS) == T and all(c % 256 == 0 for c in CHUNKS)
NQUAD = T // 512  # 4-block groups per row; one out-DMA each (q0: two)

F32 = mybir.dt.float32
F16 = mybir.dt.float16
BF16 = mybir.dt.bfloat16
U8 = mybir.dt.uint8

# eviction engine per (quad, row): the DVE stream is scans-first (the scan
# chain gates every matmul), so ACT evicts everything early plus row 0
# later, and DVE picks up row 1 from quad 3 on, once its scans are done.
# Evictions are QUAD-granular ([128, 2048] in one op): the ~260 ns fixed
# instruction cost amortizes over 4 blocks, lifting both engines' eviction
# bandwidth ~12%.  11/5 balances the two engines' finish times.
DVE_QB = {(q, 1) for q in range(3, NQUAD)}

# header column (f32): 0 = ALPHA
HC_ALPHA = 0


def build_nc(strip=True):
    nc = bass.Bass(trn_type="TRN2", target_bir_lowering=False)

    m2_d = nc.dram_tensor("m2in", [128, T], U8, kind="ExternalInput")
    hdr_d = nc.dram_tensor("hdr", [128, 4], F32, kind="ExternalInput")
    rhs_d = nc.dram_tensor("rhs", [128, D], BF16, kind="ExternalInput")
    # block-transposed layout: [row, t%128, block, d].  Each out-DMA then
    # writes 4 KiB contiguous per partition (vs 1 KiB in [row, t, d]),
    # which the DMA engines sustain at full rate.  The host untransposes.
    out = nc.dram_tensor("out", [RPC, BLK, NBLK, D], F16, kind="ExternalOutput")

    with tile.TileContext(nc) as tc:
        with (
            tc.tile_pool(name="const", bufs=1) as cpool,
            tc.tile_pool(name="psum", bufs=2, space="PSUM") as ppool,
            tc.tile_pool(name="outp", bufs=2 * NQUAD) as opool,
        ):
            m2 = cpool.tile([128, T], U8, name="m2")
            hdr_t = cpool.tile([128, 4], F32, name="hdr_t")
            rhs_t = cpool.tile([128, D], BF16, name="rhs_t")
            scr = cpool.tile([128, 8], F32, name="scr")

            # input DMAs, latency-ordered (all SP HWDGE; ~0.66 MB total);
            # the one-hot M is computed on the host (0/1 in u8, same bytes
            # as the raw indices), so the device runs no is_equal at all and
            # the m2 splits align with scan-chunk boundaries.
            nc.sync.dma_start(hdr_t[:], hdr_d[:])
            nc.sync.dma_start(m2[:, 0:256], m2_d[:, 0:256])
            nc.sync.dma_start(m2[:, 256:1024], m2_d[:, 256:1024])
            # rhs and the late m2 chunks ride the Scalar HWDGE ring: ACT is
            # idle here, and two rings triggering in parallel lands every
            # input ~1 us earlier, pulling the whole pipeline forward.
            nc.scalar.dma_start(rhs_t[:], rhs_d[:])
            nc.scalar.dma_start(m2[:, 1024:2048], m2_d[:, 1024:2048])
            nc.scalar.dma_start(m2[:, 2048:T], m2_d[:, 2048:T])

            nc.vector.memset(scr[:], 0.0)
            # observer touch: a tiny same-engine copy absorbs one DMA wait
            # so no later instruction needs two sync-wait commands.
            nc.vector.tensor_copy(scr[0:1, 1:2], hdr_t[0:1, 0:1])

            g2 = cpool.tile([128, T], BF16, name="g2")

            cs_list = [sum(CHUNKS[:i]) for i in range(len(CHUNKS) + 1)]

            def scan_chunk(c):
                cs, ce = cs_list[c], cs_list[c + 1]
                if c > 0 and cs in (256, 1024, 2048):
                    # chunk starts a fresh m2 DMA region: a tiny DVE touch
                    # absorbs the DMA wait so the scan keeps its SAME-ENGINE
                    # wait on the previous scan.  The engine does not
                    # interlock the scan's initial-carry read against the
                    # previous scan's in-flight writeback; only an explicit
                    # semaphore wait does (observed on HW as a decaying
                    # error burst at the chunk boundary).
                    nc.vector.tensor_copy(scr[0:1, 2:3], m2[0:1, cs : cs + 1])
                # G[p, t] = ALPHA * G[p, t-1] + M[p, t]; fp32 carry, bf16 out
                nc.vector.tensor_tensor_scan(
                    g2[:, cs:ce],
                    hdr_t[:, HC_ALPHA : HC_ALPHA + 1].broadcast_to((128, ce - cs)),
                    m2[:, cs:ce],
                    0.0 if c == 0 else g2[:, cs - 1 : cs],
                    mybir.AluOpType.mult,
                    mybir.AluOpType.add,
                )

            # chunk -> quad mapping: chunks 0+1 form quad 0, chunk 2 is quad
            # 1, chunks 3..5 are two quads each.  The prev-trace term
            # (alpha^(t+1) * prev, nonzero only for block 0) is added on the
            # host after the run, so no pair needs special handling here.
            # all scans up front: the chain is the serial driver of every
            # matmul, so nothing may delay it on the DVE queue.
            for c in range(len(CHUNKS)):
                scan_chunk(c)
            last_ots = []

            def do_quad(q):
                for b in range(RPC):
                    ot = opool.tile([BLK, 4 * D], F16, name="ot")
                    ps = ppool.tile([BLK, 4 * D], F32, name="ps")
                    for j in range(4):
                        k = 4 * q + j
                        nc.tensor.matmul(
                            ps[:, j * D : (j + 1) * D],
                            g2[b * V : (b + 1) * V, k * BLK : (k + 1) * BLK],
                            rhs_t[b * V : (b + 1) * V, :],
                            start=True,
                            stop=True,
                            tile_position=(b * V, 0),
                        )
                    if (q, b) not in DVE_QB:
                        nc.scalar.copy(ot[:], ps[:])
                    else:
                        nc.vector.tensor_copy(ot[:], ps[:])
                    dview = out[b, :, 4 * q : 4 * (q + 1), :]
                    sview = ot[:].rearrange("p (four d) -> p four d", four=4)
                    nc.sync.dma_start(dview, sview)
                    last_ots.append(ot[0:1, 0:1])

            for q in range(NQUAD):
                do_quad(q)

            # End-of-kernel sinks: a tiny DVE write into the read range of
            # the LAST out-DMA on each of the 8 HWDGE queues makes the DVE
            # stream observe every queue's final completion (earlier DMAs on
            # a queue are implied by FIFO), so the tail drain needs only one
            # wait after the strip pass below.
            for ap in last_ots[-8:]:
                nc.vector.tensor_copy(ap, scr[0:1, 0:1])
    if strip:
        _strip_redundant_waits(nc)
    return nc


def _strip_redundant_waits(nc):
    """Remove statically-implied semaphore waits (vector-clock analysis).

    The TRN2 instruction encodings here accept only ONE sync-wait command
    per instruction, but Tile emits extra waits for pool-slot reuse and the
    kernel-tail drain.  Many of those waits are statically implied by
    program order: engine queues execute in order, each DMA queue completes
    FIFO, and observing a semaphore value inherits every guarantee its
    updaters had.  This pass computes, for every instruction, the semaphore
    floor guaranteed at issue, and drops any wait already implied without
    it.  Straight-line (loop-free) programs only.
    """
    import concourse.mybir as mybir

    insts = []
    for fn in nc.m.functions:
        for bb in fn.blocks:
            for ins in bb.instructions:
                insts.append(ins)

    def waits(ins):
        si = ins.sync_info
        return list(si.on_wait) if si is not None else []

    def updates(ins):
        si = ins.sync_info
        return list(si.on_update) if si is not None else []

    # Streams: compute instructions execute in order per engine; a DMACopy's
    # *data completion* (its sem update) is FIFO per DMA queue, gated by its
    # trigger (engine stream) issue.
    def is_dma(ins):
        return type(ins).__name__ == "InstDMACopy"

    def dma_queue(ins):
        us = updates(ins)
        return us[0].ant_name if us else None

    # sem -> ordered list of (inst_index, add_value); single-updater-stream
    # sems only are used for transitive guarantees.
    sem_updaters = {}
    sem_streams = {}
    for i, ins in enumerate(insts):
        key = ("q", dma_queue(ins)) if is_dma(ins) else ("e", str(ins.engine))
        for u in updates(ins):
            if u.update_mode not in ("sem-inc", "sem-add-imm") or u.update_reg:
                sem_streams.setdefault(u.ant_name, set()).add("reg")
                continue
            sem_updaters.setdefault(u.ant_name, []).append((i, u.update_value))
            sem_streams.setdefault(u.ant_name, set()).add(key)

    single_stream_sems = {s for s, st in sem_streams.items() if len(st) == 1}

    # cumulative sem value right after instruction i's update
    cum_after = {}
    run = {}
    for i, ins in enumerate(insts):
        for u in updates(ins):
            if u.update_mode in ("sem-inc", "sem-add-imm") and not u.update_reg:
                run[u.ant_name] = run.get(u.ant_name, 0) + u.update_value
                cum_after[(i, u.ant_name)] = run[u.ant_name]

    prev_engine = {}
    prev_queue = {}
    last_e = {}
    last_q = {}
    for i, ins in enumerate(insts):
        ek = str(ins.engine)
        prev_engine[i] = last_e.get(ek)
        last_e[ek] = i
        if is_dma(ins):
            qk = dma_queue(ins)
            prev_queue[i] = last_q.get(qk)
            last_q[qk] = i

    n = len(insts)
    # disp[i]: sem floor guaranteed when instruction i dispatches (data-order
    # level).  done[i]: floor when its effects (sem updates) are visible —
    # for a DMACopy that is DATA completion on its queue.
    disp = [dict() for _ in range(n)]
    done = [dict() for _ in range(n)]

    def join_into(dst, src):
        changed = False
        for s, v in src.items():
            if dst.get(s, 0) < v:
                dst[s] = v
                changed = True
        return changed

    def guarantee_of_wait(sem, val):
        """Floor implied by observing sem >= val."""
        out = {sem: val}
        if sem not in single_stream_sems:
            return out
        cum = 0
        for j, add in sem_updaters.get(sem, []):
            cum += add
            join_into(out, done[j])
            if cum >= val:
                break
        return out

    def disp_floor(i, skip_wait=None):
        out = {}
        p = prev_engine[i]
        if p is not None:
            join_into(out, disp[p])
            if not is_dma(insts[p]):
                # same-engine execution is in-order: p's effects precede i's
                join_into(out, done[p])
        for w in waits(insts[i]):
            if w is skip_wait:
                continue
            if w.wait_mode == "sem-ge-imm" and not w.wait_reg:
                join_into(out, guarantee_of_wait(w.ant_name, w.wait_value))
        return out

    def recompute():
        changed = True
        while changed:
            changed = False
            for i, ins in enumerate(insts):
                f = disp_floor(i)
                if join_into(disp[i], f):
                    changed = True
                d = dict(disp[i])
                if is_dma(ins):
                    pq = prev_queue.get(i)
                    if pq is not None:
                        join_into(d, done[pq])
                for u in updates(ins):
                    c = cum_after.get((i, u.ant_name))
                    if c is not None and d.get(u.ant_name, 0) < c:
                        d[u.ant_name] = c
                if join_into(done[i], d):
                    changed = True

    recompute()
    def own_engine_sem(ins, w):
        pfx = str(ins.engine).replace("EngineType.", "")
        pfx = {"DVE": "DVE", "PE": "PE", "Activation": "Activation",
               "Pool": "Pool", "SP": "Sync"}.get(pfx, pfx)
        return w.ant_name.startswith(pfx)

    # Iteratively remove implied waits (one at a time, recomputing floors).
    # Prefer dropping cross-engine / DMA waits: a kept same-engine wait is
    # the only thing that interlocks an operand read against the previous
    # instruction's in-flight writeback (engine program order alone does
    # not — observed on HW with back-to-back scans).
    for _round in range(2000):
        victim = None
        for i, ins in enumerate(insts):
            ws = waits(ins)
            if len(ws) < 2:
                continue
            cands = []
            for w in ws:
                if w.wait_mode != "sem-ge-imm" or w.wait_reg:
                    continue
                # A DMA trigger's wait on its OWN queue's semaphore is ring
                # backpressure, not a data dependency: same-queue DMAs
                # complete FIFO regardless, and this kernel keeps well under
                # the HWDGE ring depth per queue.  Droppable.
                if is_dma(ins) and w.ant_name == dma_queue(ins):
                    cands.append((1, w))
                    continue
                f = disp_floor(i, skip_wait=w)
                if f.get(w.ant_name, 0) >= w.wait_value:
                    cands.append((0 if not own_engine_sem(ins, w) else 2, w))
            if cands:
                cands.sort(key=lambda t: t[0])
                victim = (i, cands[0][1])
                break
        if victim is None:
            break
        i, w = victim
        si = insts[i].sync_info
        kept = [x for x in si.on_wait if x is not w]
        insts[i].sync_info = mybir.SyncInfo(on_wait=kept, on_update=si.on_update)
        for d in disp:
            d.clear()
        for d in done:
            d.clear()
        recompute()

    bad = [
        (type(ins).__name__, [(w.ant_name, w.wait_value) for w in waits(ins)])
        for ins in insts
        if len(waits(ins)) >= 2
    ]
    if bad:
        raise RuntimeError(f"instructions still carry >=2 waits: {bad[:5]}")


def make_in_maps(ctrl_tokens, prev_trace, embed):
    import ml_dtypes

    bf16 = ml_dtypes.bfloat16
    idx = np.asarray(ctrl_tokens)[:, :, 1].astype(np.uint8)  # [B, T] (< 64)
    emb = np.asarray(embed, dtype=np.float32).astype(bf16)  # [V, D]
    iota = np.arange(V, dtype=np.uint8)
    in_maps = []
    for c in range(NCORES):
        rows = [RPC * c + r for r in range(RPC)]
        m2in = np.empty((128, T), np.uint8)
        for r, b in enumerate(rows):
            # one-hot: M[v, t] = (idx[b, t] == v), u8 0/1
            m2in[r * V : (r + 1) * V, :] = idx[b][None, :] == iota[:, None]
        hdr = np.zeros((128, 4), np.float32)
        hdr[:, HC_ALPHA] = ALPHA
        rhs = np.empty((128, D), bf16)
        rhs[0:V, :] = emb
        rhs[V:128, :] = emb
        in_maps.append({"m2in": m2in, "hdr": hdr, "rhs": rhs})
    return in_maps


_NC_CACHE = None


def get_nc():
    global _NC_CACHE
    if _NC_CACHE is None:
        _NC_CACHE = build_nc()
    return _NC_CACHE


def kernel(ctrl_tokens, prev_trace, embed):
    in_maps = make_in_maps(ctrl_tokens, prev_trace, embed)
    res = run_bass_kernel_spmd(get_nc(), in_maps, core_ids=list(range(NCORES)))
    # device layout is [RPC, BLK, NBLK, D]; untranspose to [B, T, D]
    out = np.concatenate([r["out"] for r in res.results], axis=0)
    out = out.transpose(0, 2, 1, 3).reshape(B, T, D)
    out = np.ascontiguousarray(out.astype(np.float32))
    # prev-trace contribution: alpha^(t+1) * prev, negligible past t=128
    prev = np.asarray(prev_trace, dtype=np.float32)
    apow = (ALPHA ** (np.arange(BLK, dtype=np.float64) + 1.0)).astype(np.float32)
    out[:, :BLK, :] += prev[:, None, :] * apow[None, :, None]
    return out


# revision 60
# speedup vs baseline: 1.1100x; 1.1100x over previous
"""EventTrace kernel for Trainium2 (8 NeuronCores, Bass/Tile).

Computes, for each batch row b:
    ev[t]   = embed[ctrl_tokens[b, t, 1]]          (gather from [64,512] table)
    c[t]    = ALPHA * c[t-1] + ev[t],  c[-1] = prev_trace[b]
    out[b]  = c                                     -> [B, T, D] float32

Algorithm (per core, 2 batch rows):
  Instead of gathering 16 MiB of embeddings, scan *decayed one-hot counts*
  G[v, t] = ALPHA * G[v, t-1] + onehot(idx_t == v) on the vector engine
  (tensor_tensor_scan keeps an fp32 carry and downcasts only the output, so
  G is written bf16 directly), then reconstruct each 128-step output block
  with one K=64 bf16 matmul per row:
      C[t, d] = sum_v G[v, t] * embed[v, d]  (+ ALPHA^(t+1) * prev[d])
  The two rows' matmuls use PE row-tiling (tile_position (0,0) / (64,0)) so
  they run concurrently.  The prev-trace carry decays below relevance after
  128 steps, so it is applied only to block 0 via a fused
  scalar_tensor_tensor during PSUM eviction.

  The kernel is output-bandwidth-bound: the full per-core result is written
  fp16 (8.4 MB instead of 16.8 MB; the fro-norm error this adds is ~3e-4,
  far inside the 2e-2 gate) and converted to f32 on the host.  PSUM is
  evicted in 2-block pairs split across DVE and ACT; the one-hot compares
  for chunks >= 1 run on GpSimd so the DVE does only scans + evictions.
  The entire fp16 output lives in SBUF (64 KiB/partition), so no out-tile
  is ever reused and no eviction waits on a previous DMA.

Sharding: batch rows across the 8 cores (2 rows per core); the embedding
table and constants are replicated.
"""

import sys

for _p in ("/root/.axon_site/_ro/trn_rl_repo", "/opt/trn_rl_repo"):
    if _p not in sys.path:
        sys.path.append(_p)

import numpy as np

import concourse.bass as bass
import concourse.tile as tile
from concourse import mybir
from concourse.bass_utils import run_bass_kernel_spmd

ALPHA = 0.9
B, T, V, D = 16, 4096, 64, 512
NCORES = 8
RPC = B // NCORES  # batch rows per core
BLK = 128
NBLK = T // BLK
# scan chunk boundaries (timesteps); first chunks small so the matmul /
# eviction / write pipeline starts as early as possible.
CHUNKS = [256, 256, 512, 1024, 1024, 1024]
assert sum(CHUNKS) == T and all(c % 256 == 0 for c in CHUNKS)
NQUAD = T // 512  # 4-block groups per row; one out-DMA each (q0: two)

F32 = mybir.dt.float32
F16 = mybir.dt.float16
BF16 = mybir.dt.bfloat16
U8 = mybir.dt.uint8

# eviction engine per (quad, row, pair): the DVE stream is scans-first (the
# scan chain gates every matmul), so ACT evicts everything early plus row 0
# later, and DVE picks up row 1 from quad 2 on, once its scans are done.
# Alternating engines WITHIN each quad keeps both engines concurrently busy
# despite the 4-slot PSUM round-robin coupling eviction order to matmul
# order.  Pair (2,1,0) stays on ACT: DVE's scan chain is still running when
# it becomes ready, and 11/21 balances the two engines' finish times.
DVE_PAIRS = {(q, 1, pj) for q in range(2, NQUAD) for pj in (0, 1)} - {(2, 1, 0)}
# quads whose two pairs ride separate DMAs (different writer engines, or
# pipeline-edge scheduling for the first/last quad)
PAIR_DMA_QUADS = (0, 2, NQUAD - 1)

# header column (f32): 0 = ALPHA
HC_ALPHA = 0


def build_nc(strip=True):
    nc = bass.Bass(trn_type="TRN2", target_bir_lowering=False)

    m2_d = nc.dram_tensor("m2in", [128, T], U8, kind="ExternalInput")
    hdr_d = nc.dram_tensor("hdr", [128, 4], F32, kind="ExternalInput")
    rhs_d = nc.dram_tensor("rhs", [128, D], BF16, kind="ExternalInput")
    # block-transposed layout: [row, t%128, block, d].  Each out-DMA then
    # writes 4 KiB contiguous per partition (vs 1 KiB in [row, t, d]),
    # which the DMA engines sustain at full rate.  The host untransposes.
    out = nc.dram_tensor("out", [RPC, BLK, NBLK, D], F16, kind="ExternalOutput")

    with tile.TileContext(nc) as tc:
        with (
            tc.tile_pool(name="const", bufs=1) as cpool,
            tc.tile_pool(name="psum", bufs=4, space="PSUM") as ppool,
            tc.tile_pool(name="outp", bufs=2 * NQUAD) as opool,
        ):
            m2 = cpool.tile([128, T], U8, name="m2")
            hdr_t = cpool.tile([128, 4], F32, name="hdr_t")
            rhs_t = cpool.tile([128, D], BF16, name="rhs_t")
            scr = cpool.tile([128, 8], F32, name="scr")

            # input DMAs, latency-ordered (all SP HWDGE; ~0.66 MB total);
            # the one-hot M is computed on the host (0/1 in u8, same bytes
            # as the raw indices), so the device runs no is_equal at all and
            # the m2 splits align with scan-chunk boundaries.
            nc.sync.dma_start(hdr_t[:], hdr_d[:])
            nc.sync.dma_start(m2[:, 0:256], m2_d[:, 0:256])
            nc.sync.dma_start(m2[:, 256:1024], m2_d[:, 256:1024])
            # rhs and the late m2 chunks ride the Scalar HWDGE ring: ACT is
            # idle here, and two rings triggering in parallel lands every
            # input ~1 us earlier, pulling the whole pipeline forward.
            nc.scalar.dma_start(rhs_t[:], rhs_d[:])
            nc.scalar.dma_start(m2[:, 1024:2048], m2_d[:, 1024:2048])
            nc.scalar.dma_start(m2[:, 2048:T], m2_d[:, 2048:T])

            nc.vector.memset(scr[:], 0.0)
            # observer touch: a tiny same-engine copy absorbs one DMA wait
            # so no later instruction needs two sync-wait commands.
            nc.vector.tensor_copy(scr[0:1, 1:2], hdr_t[0:1, 0:1])

            g2 = cpool.tile([128, T], BF16, name="g2")

            cs_list = [sum(CHUNKS[:i]) for i in range(len(CHUNKS) + 1)]

            def scan_chunk(c):
                cs, ce = cs_list[c], cs_list[c + 1]
                if c > 0 and cs in (256, 1024, 2048):
                    # chunk starts a fresh m2 DMA region: a tiny DVE touch
                    # absorbs the DMA wait so the scan keeps its SAME-ENGINE
                    # wait on the previous scan.  The engine does not
                    # interlock the scan's initial-carry read against the
                    # previous scan's in-flight writeback; only an explicit
                    # semaphore wait does (observed on HW as a decaying
                    # error burst at the chunk boundary).
                    nc.vector.tensor_copy(scr[0:1, 2:3], m2[0:1, cs : cs + 1])
                # G[p, t] = ALPHA * G[p, t-1] + M[p, t]; fp32 carry, bf16 out
                nc.vector.tensor_tensor_scan(
                    g2[:, cs:ce],
                    hdr_t[:, HC_ALPHA : HC_ALPHA + 1].broadcast_to((128, ce - cs)),
                    m2[:, cs:ce],
                    0.0 if c == 0 else g2[:, cs - 1 : cs],
                    mybir.AluOpType.mult,
                    mybir.AluOpType.add,
                )

            # chunk -> quad mapping: chunks 0+1 form quad 0, chunk 2 is quad
            # 1, chunks 3..5 are two quads each.  The prev-trace term
            # (alpha^(t+1) * prev, nonzero only for block 0) is added on the
            # host after the run, so no pair needs special handling here.
            # all scans up front: the chain is the serial driver of every
            # matmul, so nothing may delay it on the DVE queue.
            for c in range(len(CHUNKS)):
                scan_chunk(c)
            last_ots = []

            def do_quad(q):
                for b in range(RPC):
                    ot = opool.tile([BLK, 4 * D], F16, name="ot")
                    for pairj in range(2):
                        ps = ppool.tile([BLK, 2 * D], F32, name="ps")
                        for h in range(2):
                            k = 4 * q + 2 * pairj + h
                            nc.tensor.matmul(
                                ps[:, h * D : (h + 1) * D],
                                g2[b * V : (b + 1) * V, k * BLK : (k + 1) * BLK],
                                rhs_t[b * V : (b + 1) * V, :],
                                start=True,
                                stop=True,
                                tile_position=(b * V, 0),
                            )
                        dst = ot[:, pairj * 2 * D : (pairj + 1) * 2 * D]
                        if (q, b, pairj) not in DVE_PAIRS:
                            nc.scalar.copy(dst, ps[:])
                        else:
                            nc.vector.tensor_copy(dst, ps[:])
                    # one out-DMA per quad; edge quads go as two pair-DMAs
                    # (first: write stream starts one eviction earlier;
                    # last: the tail drains two queues; q2: its two pairs
                    # have different writer engines)
                    if q in PAIR_DMA_QUADS:
                        for pairj in range(2):
                            dview = out[b, :, 4 * q + 2 * pairj : 4 * q + 2 * pairj + 2, :]
                            sview = ot[
                                :, pairj * 2 * D : (pairj + 1) * 2 * D
                            ].rearrange("p (two d) -> p two d", two=2)
                            nc.sync.dma_start(dview, sview)
                            # each pair-DMA needs its own end-of-kernel sink
                            last_ots.append(
                                ot[0:1, pairj * 2 * D : pairj * 2 * D + 1]
                            )
                    else:
                        dview = out[b, :, 4 * q : 4 * (q + 1), :]
                        sview = ot[:].rearrange("p (four d) -> p four d", four=4)
                        nc.sync.dma_start(dview, sview)
                        last_ots.append(ot[0:1, 0:1])

            for q in range(NQUAD):
                do_quad(q)

            # End-of-kernel sinks: a tiny DVE write into the read range of
            # the LAST out-DMA on each of the 8 HWDGE queues makes the DVE
            # stream observe every queue's final completion (earlier DMAs on
            # a queue are implied by FIFO), so the tail drain needs only one
            # wait after the strip pass below.
            for ap in last_ots[-8:]:
                nc.vector.tensor_copy(ap, scr[0:1, 0:1])
    if strip:
        _strip_redundant_waits(nc)
    return nc


def _strip_redundant_waits(nc):
    """Remove statically-implied semaphore waits (vector-clock analysis).

    The TRN2 instruction encodings here accept only ONE sync-wait command
    per instruction, but Tile emits extra waits for pool-slot reuse and the
    kernel-tail drain.  Many of those waits are statically implied by
    program order: engine queues execute in order, each DMA queue completes
    FIFO, and observing a semaphore value inherits every guarantee its
    updaters had.  This pass computes, for every instruction, the semaphore
    floor guaranteed at issue, and drops any wait already implied without
    it.  Straight-line (loop-free) programs only.
    """
    import concourse.mybir as mybir

    insts = []
    for fn in nc.m.functions:
        for bb in fn.blocks:
            for ins in bb.instructions:
                insts.append(ins)

    def waits(ins):
        si = ins.sync_info
        return list(si.on_wait) if si is not None else []

    def updates(ins):
        si = ins.sync_info
        return list(si.on_update) if si is not None else []

    # Streams: compute instructions execute in order per engine; a DMACopy's
    # *data completion* (its sem update) is FIFO per DMA queue, gated by its
    # trigger (engine stream) issue.
    def is_dma(ins):
        return type(ins).__name__ == "InstDMACopy"

    def dma_queue(ins):
        us = updates(ins)
        return us[0].ant_name if us else None

    # sem -> ordered list of (inst_index, add_value); single-updater-stream
    # sems only are used for transitive guarantees.
    sem_updaters = {}
    sem_streams = {}
    for i, ins in enumerate(insts):
        key = ("q", dma_queue(ins)) if is_dma(ins) else ("e", str(ins.engine))
        for u in updates(ins):
            if u.update_mode not in ("sem-inc", "sem-add-imm") or u.update_reg:
                sem_streams.setdefault(u.ant_name, set()).add("reg")
                continue
            sem_updaters.setdefault(u.ant_name, []).append((i, u.update_value))
            sem_streams.setdefault(u.ant_name, set()).add(key)

    single_stream_sems = {s for s, st in sem_streams.items() if len(st) == 1}

    # cumulative sem value right after instruction i's update
    cum_after = {}
    run = {}
    for i, ins in enumerate(insts):
        for u in updates(ins):
            if u.update_mode in ("sem-inc", "sem-add-imm") and not u.update_reg:
                run[u.ant_name] = run.get(u.ant_name, 0) + u.update_value
                cum_after[(i, u.ant_name)] = run[u.ant_name]

    prev_engine = {}
    prev_queue = {}
    last_e = {}
    last_q = {}
    for i, ins in enumerate(insts):
        ek = str(ins.engine)
        prev_engine[i] = last_e.get(ek)
        last_e[ek] = i
        if is_dma(ins):
            qk = dma_queue(ins)
            prev_queue[i] = last_q.get(qk)
            last_q[qk] = i

    n = len(insts)
    # disp[i]: sem floor guaranteed when instruction i dispatches (data-order
    # level).  done[i]: floor when its effects (sem updates) are visible —
    # for a DMACopy that is DATA completion on its queue.
    disp = [dict() for _ in range(n)]
    done = [dict() for _ in range(n)]

    def join_into(dst, src):
        changed = False
        for s, v in src.items():
            if dst.get(s, 0) < v:
                dst[s] = v
                changed = True
        return changed

    def guarantee_of_wait(sem, val):
        """Floor implied by observing sem >= val."""
        out = {sem: val}
        if sem not in single_stream_sems:
            return out
        cum = 0
        for j, add in sem_updaters.get(sem, []):
            cum += add
            join_into(out, done[j])
            if cum >= val:
                break
        return out

    def disp_floor(i, skip_wait=None):
        out = {}
        p = prev_engine[i]
        if p is not None:
            join_into(out, disp[p])
            if not is_dma(insts[p]):
                # same-engine execution is in-order: p's effects precede i's
                join_into(out, done[p])
        for w in waits(insts[i]):
            if w is skip_wait:
                continue
            if w.wait_mode == "sem-ge-imm" and not w.wait_reg:
                join_into(out, guarantee_of_wait(w.ant_name, w.wait_value))
        return out

    def recompute():
        changed = True
        while changed:
            changed = False
            for i, ins in enumerate(insts):
                f = disp_floor(i)
                if join_into(disp[i], f):
                    changed = True
                d = dict(disp[i])
                if is_dma(ins):
                    pq = prev_queue.get(i)
                    if pq is not None:
                        join_into(d, done[pq])
                for u in updates(ins):
                    c = cum_after.get((i, u.ant_name))
                    if c is not None and d.get(u.ant_name, 0) < c:
                        d[u.ant_name] = c
                if join_into(done[i], d):
                    changed = True

    recompute()
    def own_engine_sem(ins, w):
        pfx = str(ins.engine).replace("EngineType.", "")
        pfx = {"DVE": "DVE", "PE": "PE", "Activation": "Activation",
               "Pool": "Pool", "SP": "Sync"}.get(pfx, pfx)
        return w.ant_name.startswith(pfx)

    # Iteratively remove implied waits (one at a time, recomputing floors).
    # Prefer dropping cross-engine / DMA waits: a kept same-engine wait is
    # the only thing that interlocks an operand read against the previous
    # instruction's in-flight writeback (engine program order alone does
    # not — observed on HW with back-to-back scans).
    for _round in range(2000):
        victim = None
        for i, ins in enumerate(insts):
            ws = waits(ins)
            if len(ws) < 2:
                continue
            cands = []
            for w in ws:
                if w.wait_mode != "sem-ge-imm" or w.wait_reg:
                    continue
                # A DMA trigger's wait on its OWN queue's semaphore is ring
                # backpressure, not a data dependency: same-queue DMAs
                # complete FIFO regardless, and this kernel keeps well under
                # the HWDGE ring depth per queue.  Droppable.
                if is_dma(ins) and w.ant_name == dma_queue(ins):
                    cands.append((1, w))
                    continue
                f = disp_floor(i, skip_wait=w)
                if f.get(w.ant_name, 0) >= w.wait_value:
                    cands.append((0 if not own_engine_sem(ins, w) else 2, w))
            if cands:
                cands.sort(key=lambda t: t[0])
                victim = (i, cands[0][1])
                break
        if victim is None:
            break
        i, w = victim
        si = insts[i].sync_info
        kept = [x for x in si.on_wait if x is not w]
        insts[i].sync_info = mybir.SyncInfo(on_wait=kept, on_update=si.on_update)
        for d in disp:
            d.clear()
        for d in done:
            d.clear()
        recompute()

    bad = [
        (type(ins).__name__, [(w.ant_name, w.wait_value) for w in waits(ins)])
        for ins in insts
        if len(waits(ins)) >= 2
    ]
    if bad:
        raise RuntimeError(f"instructions still carry >=2 waits: {bad[:5]}")


def make_in_maps(ctrl_tokens, prev_trace, embed):
    import ml_dtypes

    bf16 = ml_dtypes.bfloat16
    idx = np.asarray(ctrl_tokens)[:, :, 1].astype(np.uint8)  # [B, T] (< 64)
    emb = np.asarray(embed, dtype=np.float32).astype(bf16)  # [V, D]
    iota = np.arange(V, dtype=np.uint8)
    in_maps = []
    for c in range(NCORES):
        rows = [RPC * c + r for r in range(RPC)]
        m2in = np.empty((128, T), np.uint8)
        for r, b in enumerate(rows):
            # one-hot: M[v, t] = (idx[b, t] == v), u8 0/1
            m2in[r * V : (r + 1) * V, :] = idx[b][None, :] == iota[:, None]
        hdr = np.zeros((128, 4), np.float32)
        hdr[:, HC_ALPHA] = ALPHA
        rhs = np.empty((128, D), bf16)
        rhs[0:V, :] = emb
        rhs[V:128, :] = emb
        in_maps.append({"m2in": m2in, "hdr": hdr, "rhs": rhs})
    return in_maps


_NC_CACHE = None


def get_nc():
    global _NC_CACHE
    if _NC_CACHE is None:
        _NC_CACHE = build_nc()
    return _NC_CACHE


def kernel(ctrl_tokens, prev_trace, embed):
    in_maps = make_in_maps(ctrl_tokens, prev_trace, embed)
    res = run_bass_kernel_spmd(get_nc(), in_maps, core_ids=list(range(NCORES)))
    # device layout is [RPC, BLK, NBLK, D]; untranspose to [B, T, D]
    out = np.concatenate([r["out"] for r in res.results], axis=0)
    out = out.transpose(0, 2, 1, 3).reshape(B, T, D)
    out = np.ascontiguousarray(out.astype(np.float32))
    # prev-trace contribution: alpha^(t+1) * prev, negligible past t=128
    prev = np.asarray(prev_trace, dtype=np.float32)
    apow = (ALPHA ** (np.arange(BLK, dtype=np.float64) + 1.0)).astype(np.float32)
    out[:, :BLK, :] += prev[:, None, :] * apow[None, :, None]
    return out
